# revision 1
# baseline (speedup 1.0000x reference)
"""Trainium2 Bass kernel for nn_BaseRuleLearner.

Math (per batch element b, reference semantics):
  UM[b,i,v,l]      = sum_e U[b,l,e]  * ru[i,v,e]
  BM[b,i,n,m,j,k]  = sum_e Bf[b,j,k,e] * rb[i,n,m,e]
  scores[b,i,p]    = sum_v UM[b,i,v,perm[p,v]]
                   + sum_{n,m} BM[b,i,n,m,perm[p,n],perm[p,m]]
  merged[b,i]      = min_p scores[b,i,p]
  out[b,:]         = softmax_i(merged) @ one_hot([0,0,1,1])

Kernel strategy (pure data parallel over B across 8 cores, 512 b/core).
DMA instruction count is the scarce resource (~625ns serialized HWDGE
overhead per dma_start), so everything is packed into few, large,
rectangular transfers:

Stage 1 (PE, float32r): one matmul per jk-PAIR jp (jk = 2*jp+s); a
block-diagonal weight [128=(s,e), 72=(i, t=s*9+nm)] packs the two k=64
contractions of a pair into one k=128 matmul:
psum[72=(i,t), 512 b] = BM[b,i,nm, jk=2jp+s].  Same for unary
(l = 2*lp+s, rows (i, tu=s*3+v)).

Evac (DVE/ACT alternating): psum -> SG staging [72, (jp, b)] in SBUF.

Assembly (1 DMA per (chunk, i), 24 total): Q-row order r = t*32 + jp
(unary r = 576 + tu*4 + lp) makes SG[i*18+4c : +nt, :] and
qt[c][0:nt*32, i*512:+512] the same element stream: src iterates
(t, jp, b), dst iterates (row=t*32+jp, b).

Stage 2 (PE, float32r): scores^T per (i, b-tile): psum[128 b, 336 p]
accumulated over 5 k-chunks; lhsT = qt[c] slice (stationary), rhs = G
chunk [k, 336], the 0/1 permutation-gather matrix (host-built,
input-independent).

Final: DVE min over p (free axis), softmax over i=4 (free axis),
pair-sum into [128, 4] result tiles, one gathered DMA out.
"""

import itertools
import numpy as np

B, O, E = 4096, 8, 64
I, V = 4, 3
P = 336
N_CORES = 8
BC = B // N_CORES            # 512 batch per core
NJP = (O * O) // 2           # 32 jk-pairs
NLP = O // 2                 # 4 l-pairs
R_ND = 12 * 32               # off-diag rows of Q/G (t'' major, jp minor)
R_DG = 24                    # diag rows: (s,nmd) x 4 used jp
R_UN = 24                    # unary rows
R_TOT = R_ND + R_DG + R_UN   # 456 total rows
K_CHUNKS = [(0, 128), (128, 256), (256, 384), (384, R_TOT)]
OD_IX = {1: 0, 2: 1, 3: 2, 5: 3, 6: 4, 7: 5}   # offdiag nm -> 0..5
DG_IX = {0: 0, 4: 1, 8: 2}                     # diag nm -> 0..2
NBT = BC // 128              # b-tiles per core (4)
JBS = BC + 16                # padded jp-block stride in sg (separate DMA runs)
JPG = 4                      # jp's per input DMA group
NXG = NJP // JPG             # binary input groups (4)

_PERM = np.array(list(itertools.permutations(range(O), V)), dtype=np.int32)

_CACHED = {}


def _build_g_packed():
    """G[r, p] in the pruned, t''-major layout:
    off-diag rows r = (s*6 + OD_IX[nm])*32 + jp for jk=2jp+s=j*8+k;
    diag rows (j==k only) r = 384 + (s*3 + DG_IX[nm])*4 + jpi;
    unary rows r = 408 + (s*3 + v)*4 + lp (l=2lp+s).
    Packed into [128, 4*336]: col-block c holds G rows [128c : 128c+kc]."""
    g = np.zeros((R_TOT, P), np.float32)
    ar = np.arange(P)
    for n in range(V):
        for m in range(V):
            nm = n * V + m
            jk = _PERM[:, n] * O + _PERM[:, m]
            s, jp = jk % 2, jk // 2
            if nm in OD_IX:
                r = (s * 6 + OD_IX[nm]) * NJP + jp
                g[r, ar] = 1.0
            else:
                mask = _PERM[:, n] == _PERM[:, m]
                jpi = (jp[mask] - 4 * s[mask]) // 9
                r = R_ND + (s[mask] * 3 + DG_IX[nm]) * 4 + jpi
                g[r, ar[mask]] = 1.0
    for v in range(V):
        l = _PERM[:, v]
        r = R_ND + R_DG + (l % 2 * V + v) * NLP + l // 2
        g[r, ar] = 1.0
    packed = np.zeros((128, len(K_CHUNKS) * P), np.float32)
    for c, (r0, r1) in enumerate(K_CHUNKS):
        packed[0 : r1 - r0, c * P : (c + 1) * P] = g[r0:r1]
    return packed


def _build_module():
    import concourse.tile as tile
    from concourse import bacc, mybir

    FP = mybir.dt.float32
    FR = mybir.dt.float32r
    BF = mybir.dt.bfloat16
    X = mybir.AxisListType.X
    nc = bacc.Bacc("TRN2", target_bir_lowering=False, debug=False)

    ab = nc.dram_tensor("ab", [128, NJP * BC], BF, kind="ExternalInput")
    au = nc.dram_tensor("au", [128, NLP * BC], BF, kind="ExternalInput")
    w = nc.dram_tensor("w", [128, 96], BF, kind="ExternalInput")
    gm = nc.dram_tensor("gm", [128, len(K_CHUNKS) * P], BF, kind="ExternalInput")
    out = nc.dram_tensor("out", [BC, 4], FP, kind="ExternalOutput")

    with tile.TileContext(nc) as tc:
        with (
            tc.tile_pool(name="wpool", bufs=1) as wpool,
            tc.tile_pool(name="xpool", bufs=3) as xpool,
            tc.tile_pool(name="sgpool", bufs=1) as sgpool,
            tc.tile_pool(name="qpool", bufs=1) as qpool,
            tc.tile_pool(name="mpool", bufs=2) as mpool,
            tc.tile_pool(name="psb", bufs=3, space="PSUM") as psb,
            tc.tile_pool(name="psu", bufs=1, space="PSUM") as psu,
            tc.tile_pool(name="pss", bufs=4, space="PSUM") as pss,
        ):
            # ---- phase 0: weights + G (one DMA each) ----
            w_sb = wpool.tile([128, 96], BF, tag="w")
            nc.sync.dma_start(w_sb[:], w.ap()[:])
            rb_sb = w_sb[:, 0:72]
            ru_sb = w_sb[:, 72:96]
            g_sb = wpool.tile([128, len(K_CHUNKS) * P], BF, tag="g")
            nc.sync.dma_start(g_sb[:], gm.ap()[:])

            qt = [
                [
                    qpool.tile(
                        [128, BC], BF, tag=f"q{c}_{i}", name=f"q{c}_{i}"
                    )
                    for i in range(I)
                ]
                for c in range(4)
            ]
            sg = sgpool.tile([72, NJP * JBS], BF, tag="sg")
            sgu = sgpool.tile([24, NLP * JBS], BF, tag="sgu")

            # ---- phase 1u: unary ----
            xu = xpool.tile([128, NLP * BC], BF, tag="xu")
            nc.sync.dma_start(xu[:], au.ap()[:])
            for lp in range(NLP):
                pu = psu.tile([24, BC], FP, tag="pu")
                nc.tensor.matmul(
                    pu[:],
                    ru_sb,
                    xu[:, lp * BC : (lp + 1) * BC],
                    start=True,
                    stop=True,
                )
                nc.vector.tensor_copy(sgu[:, lp * JBS : lp * JBS + BC], pu[:])

            # ---- phase 1: binary stage-1 matmuls + evac ----
            for xg in range(NXG):
                xt = xpool.tile([128, JPG * BC], BF, tag="x")
                ieng = nc.sync
                ieng.dma_start(
                    xt[:], ab.ap()[:, xg * JPG * BC : (xg + 1) * JPG * BC]
                )
                for jl in range(JPG):
                    jp = xg * JPG + jl
                    pb = psb.tile([72, BC], FP, tag="pb")
                    nc.tensor.matmul(
                        pb[:],
                        rb_sb,
                        xt[:, jl * BC : (jl + 1) * BC],
                        start=True,
                        stop=True,
                    )
                    dst = sg[:, jp * JBS : jp * JBS + BC]
                    if jp % 2 == 0:
                        nc.vector.tensor_copy(dst, pb[:])
                    else:
                        nc.scalar.copy(dst, pb[:])

            # ---- assembly: 1 DMA per (chunk, i); padded src runs ----
            for i in range(I):
                srcvu = (
                    sgu[i * 6 : i * 6 + 6, :]
                    .rearrange("p (a m) -> p a m", m=JBS)[:, :, 0:BC]
                )
                nc.sync.dma_start(qt[3][i][R_DG : R_DG + R_UN, :], srcvu)
            for i in range(I):
                for s in range(2):
                    # diag rows: src t'' = 12 + s*3 .. +3, jp in {4s, 4s+9, ...}
                    srcd = (
                        sg[i * 18 + 12 + s * 3 : i * 18 + 12 + s * 3 + 3, :]
                        .rearrange("p (a m) -> p a m", m=JBS)
                        [:, 4 * s : 4 * s + 28 : 9, 0:BC]
                    )
                    nc.sync.dma_start(
                        qt[3][i][s * 12 : s * 12 + 12, :], srcd
                    )
                for c in range(3):
                    srcv = (
                        sg[i * 18 + 4 * c : i * 18 + 4 * c + 4, :]
                        .rearrange("p (a m) -> p a m", m=JBS)[:, :, 0:BC]
                    )
                    nc.gpsimd.dma_start(qt[c][i][:, :], srcv)

            # ---- phase 2: scores + min + softmax ----
            fin = mpool.tile([128, 4 * NBT], FP, tag="fin", bufs=1)
            for bt in range(NBT):
                merged = mpool.tile([128, 4], FP, tag="m")
                for i in range(I):
                    sc = pss.tile([128, P], FP, tag="sc")
                    col = bt * 128
                    for c, (r0, r1) in enumerate(K_CHUNKS):
                        kc = r1 - r0
                        nc.tensor.matmul(
                            sc[:],
                            qt[c][i][0:kc, col : col + 128],
                            g_sb[0:kc, c * P : (c + 1) * P],
                            start=(c == 0),
                            stop=(c == len(K_CHUNKS) - 1),
                        )
                    nc.vector.tensor_reduce(
                        merged[:, i : i + 1], sc[:], axis=X, op=mybir.AluOpType.min
                    )
                mx = mpool.tile([128, 1], FP, tag="mx")
                nc.vector.tensor_reduce(
                    mx[:], merged[:], axis=X, op=mybir.AluOpType.max
                )
                sh = mpool.tile([128, 4], FP, tag="sh")
                nc.vector.tensor_scalar_sub(sh[:], merged[:], mx[:])
                ex = mpool.tile([128, 4], FP, tag="ex")
                sm = mpool.tile([128, 1], FP, tag="sm")
                nc.scalar.activation(
                    ex[:], sh[:], mybir.ActivationFunctionType.Exp, accum_out=sm[:]
                )
                rc = mpool.tile([128, 1], FP, tag="rc")
                nc.vector.reciprocal(rc[:], sm[:])
                pr = mpool.tile([128, 4], FP, tag="pr")
                nc.vector.tensor_scalar_mul(pr[:], ex[:], rc[:])
                pr3 = pr[:].rearrange("p (a b) -> p a b", b=2)
                nc.vector.tensor_add(
                    fin[:, bt * 4 : bt * 4 + 2], pr3[:, :, 0], pr3[:, :, 1]
                )
                nc.vector.memset(fin[:, bt * 4 + 2 : bt * 4 + 4], 0.0)
            # single gathered output DMA: out[bt*128 + q, col] = fin[q, bt*4+col]
            outv = out.ap().rearrange("(a p) m -> p a m", p=128)  # [128, NBT, 4]
            nc.sync.dma_start(outv, fin[:].rearrange("p (a m) -> p a m", a=NBT))

    nc.compile()
    return nc


def _get_module():
    if "nc" not in _CACHED:
        _CACHED["nc"] = _build_module()
    return _CACHED["nc"]


def _host_inputs(unary_feats, binary_feats, rule_unary, rule_binary):
    """Shard + lay out inputs for the 8 cores."""
    import ml_dtypes

    bf16 = ml_dtypes.bfloat16
    uf = np.asarray(unary_feats, dtype=np.float32).astype(bf16)
    bf = np.asarray(binary_feats, dtype=np.float32).astype(bf16)
    ru = np.asarray(rule_unary, dtype=np.float32).astype(bf16)
    rb = np.asarray(rule_binary, dtype=np.float32).astype(bf16)

    rb_flat = rb.transpose(3, 0, 1, 2).reshape(E, I * 9)   # [e, (i,nm)]
    ru_flat = ru.transpose(2, 0, 1).reshape(E, I * V)      # [e, (i,v)]
    w = np.zeros((128, 96), bf16)
    for s in range(2):
        for i in range(I):
            for nm in range(9):
                t2 = s * 6 + OD_IX[nm] if nm in OD_IX else 12 + s * 3 + DG_IX[nm]
                w[s * 64 : (s + 1) * 64, i * 18 + t2] = rb_flat[:, i * 9 + nm]
            w[s * 64 : (s + 1) * 64, 72 + i * 6 + s * 3 : 72 + i * 6 + s * 3 + 3] = (
                ru_flat[:, i * 3 : (i + 1) * 3]
            )
    g = _build_g_packed().astype(bf16)

    in_maps = []
    for c in range(N_CORES):
        bfc = bf[c * BC : (c + 1) * BC]                    # [BC, O, O, E]
        x = bfc.reshape(BC, O * O, E).transpose(1, 2, 0)   # [jk, e, b]
        ab = np.ascontiguousarray(
            x.reshape(NJP, 2, E, BC).transpose(1, 2, 0, 3)
        ).reshape(128, NJP * BC)                           # [(s,e), (jp,b)]
        ufc = uf[c * BC : (c + 1) * BC]                    # [BC, O, E]
        xu = ufc.transpose(1, 2, 0)                        # [l, e, b]
        au = np.ascontiguousarray(
            xu.reshape(NLP, 2, E, BC).transpose(1, 2, 0, 3)
        ).reshape(128, NLP * BC)                           # [(s,e), (lp,b)]
        in_maps.append({"ab": ab, "au": au, "w": w, "gm": g})
    return in_maps


TRACE = False  # set True (e.g. from test.py) to capture an NTFF profile


def kernel(unary_feats, binary_feats, rule_unary, rule_binary):
    from concourse.bass_utils import run_bass_kernel_spmd

    nc = _get_module()
    in_maps = _host_inputs(unary_feats, binary_feats, rule_unary, rule_binary)
    res = run_bass_kernel_spmd(
        nc, in_maps, core_ids=list(range(N_CORES)), trace=TRACE
    )
    _CACHED["last_results"] = res
    return np.concatenate(
        [res.results[c]["out"] for c in range(N_CORES)], axis=0
    )



# revision 10
# speedup vs baseline: 1.4789x; 1.4789x over previous
"""Trainium2 Bass kernel for nn_BaseRuleLearner (compact pair-packed design).

Math (per batch element b, reference semantics):
  UM[b,i,v,l]      = sum_e U[b,l,e]  * ru[i,v,e]
  BM[b,i,n,m,j,k]  = sum_e Bf[b,j,k,e] * rb[i,n,m,e]
  scores[b,i,p]    = sum_v UM[b,i,v,perm[p,v]]
                   + sum_{n,m} BM[b,i,n,m,perm[p,n],perm[p,m]]
  merged[b,i]      = min_p scores[b,i,p]
  out[b,:]         = softmax_i(merged) @ one_hot([0,0,1,1])

Packing (pure data parallel over B across 8 cores, BC=512 b/core):

Offdiag: the 6 ordered (n,m) n!=m gather terms pair up: for unordered
variable pair u={n<m} and unordered object pair {j<k} (28 pairs "jp"),
  CM[b,i,u,d,jp] = BM[b,i,n,m,j',k'] + BM[b,i,m,n,k',j']
with (j',k') = (j,k) if d==0 else (k,j).  Input column (jp,b) stacks
Bf[b,j,k,:] (kappa 0:64) and Bf[b,k,j,:] (kappa 64:128), so one k=128
matmul per jp computes all 24 = (ud,i) outputs; weight wb[128,32]
(24 real cols + 8 zero pad).  Each permutation p then needs only 3
offdiag terms (one per u) instead of 6.

Diag+unary fold: column (l,b) stacks U[b,l,:] and Bf[b,l,l,:];
weight wu2 col (v,i) stacks ru[i,v,:] and rb[i,v,v,:], so
  U2M[b,i,v,l] = UM[b,i,v,l] + BM[b,i,v,v,l,l]
covers unary + diagonal binary terms in one k=128 matmul per l.

PSUM packing: 3 stage-1 outputs (32 rows each: (ud,i), ud zero-padded
to 8) per bank via matmul tile_position col offsets {0,32,64} (96 is
quadrant 3 = unusable); one [96,512] evac copy per bank.  Slots that
no real matmul writes get dummy matmuls so no pre-kernel PSUM NaN can
reach stage-2.

Assembly (i -> columns, (l,ud,jp) -> k-rows) goes through DRAM,
where the partition-boundary reinterpretation is free (SBUF->SBUF
DMAs with >=2 partition dims on the source scramble data - HW
descriptor-pairing bug): hop1 scatters sg[96, (g,b)] -> scratch
DRAM[g, p*512+b] (trivial APs); hop2 reloads scratch viewed flat as
[24g+lud, (i,b)] -> qt (plain copy).  RAW through DRAM is not
dep-tracked, so hop2 gets a forced dep via set_after_insts.
k-chunks: chunk0 [120 rows] = jp 0..14, chunk1 [120] = jp 15..27
(+junk), chunk2 [72] = unary' (v padded to 8); ud padded to 8 keeps
the flat view affine.
Stage-2: psum[128 b, 336 p] accumulated over 3 matmuls vs G chunks
(0/1 gather matrix; junk k-rows have all-zero G rows).  Fused DVE
tensor_tensor_reduce does min(168 vs 168) + reduce in one pass.
"""

import itertools
import numpy as np

B, O, E = 4096, 8, 64
I, V = 4, 3
P = 336
N_CORES = 8
BC = B // N_CORES            # 512 batch per core
NP = 28                      # unordered offdiag object pairs
NBT = BC // 128              # b-tiles per core (4)
NCH = 4                      # ab input DMA chunks (7 jp each)

_PERM = np.array(list(itertools.permutations(range(O), V)), dtype=np.int32)
_PAIRS = [(j, k) for j in range(O) for k in range(j + 1, O)]
_PIDX = np.full((O, O), -1, np.int32)
for _idx, (_j, _k) in enumerate(_PAIRS):
    _PIDX[_j, _k] = _idx
_PAIRS3 = [(0, 1), (0, 2), (1, 2)]

_CACHED = {}


def _build_g():
    """G0[120,P] (jp 0..14: row 24*(jp//3) + 8*(jp%3) + ud),
    G1[120,P] (jp 15..26 same with jp-15; jp27: row 96+ud),
    G2[72,P] (unary': row 24*h + 8*ms + v, l = 3h+ms)."""
    g0 = np.zeros((120, P), np.float32)
    g1 = np.zeros((120, P), np.float32)
    g2 = np.zeros((72, P), np.float32)
    for u, (n, m) in enumerate(_PAIRS3):
        a = _PERM[:, n]
        c = _PERM[:, m]
        j = np.minimum(a, c)
        k = np.maximum(a, c)
        jp = _PIDX[j, k]
        d = (a > c).astype(np.int32)
        ud = u * 2 + d
        for p in range(P):
            jpp = int(jp[p])
            udp = int(ud[p])
            if jpp < 15:
                g0[24 * (jpp // 3) + 8 * (jpp % 3) + udp, p] = 1.0
            elif jpp < 27:
                g1[24 * ((jpp - 15) // 3) + 8 * ((jpp - 15) % 3) + udp, p] = 1.0
            else:
                g1[96 + udp, p] = 1.0
    for v in range(V):
        for p in range(P):
            l = int(_PERM[p, v])
            h = l // 3 if l < 6 else 2
            ms = l - 3 * h
            g2[24 * h + 8 * ms + v, p] = 1.0
    return g0, g1, g2


def _build_module():
    import concourse.tile as tile
    from concourse import bacc, mybir

    FP = mybir.dt.float32
    BF = mybir.dt.bfloat16
    MIN = mybir.AluOpType.min
    nc = bacc.Bacc("TRN2", target_bir_lowering=False, debug=False)

    ab = nc.dram_tensor("ab", [128, NP * BC], BF, kind="ExternalInput")
    au = nc.dram_tensor("au", [128, O * BC], BF, kind="ExternalInput")
    wg = nc.dram_tensor("wg", [128, 64 + 3 * P], BF, kind="ExternalInput")
    out = nc.dram_tensor("out", [BC, 4], FP, kind="ExternalOutput")
    scrA = nc.dram_tensor("scrA", [5, 96 * BC], BF, kind="Internal")
    scrB = nc.dram_tensor("scrB", [5, 96 * BC], BF, kind="Internal")
    scrU = nc.dram_tensor("scrU", [3, 96 * BC], BF, kind="Internal")

    CPJ = NP // NCH          # jp per input chunk (7)

    with tile.TileContext(nc) as tc:
        with (
            tc.tile_pool(name="wpool", bufs=1) as wpool,
            tc.tile_pool(name="mpool", bufs=2) as mpool,
            tc.tile_pool(name="psb", bufs=3, space="PSUM") as psb,
            tc.tile_pool(name="psu", bufs=2, space="PSUM") as psu,
            tc.tile_pool(name="pss", bufs=3, space="PSUM") as pss,
        ):
            # ---- inputs ----
            wg_sb = wpool.tile([128, 64 + 3 * P], BF, tag="wg")
            nc.sync.dma_start(wg_sb[:], wg.ap()[:])
            au_sb = wpool.tile([128, O * BC], BF, tag="au")
            nc.sync.dma_start(au_sb[:], au.ap()[:])
            ab_sb = []
            for c in range(NCH):
                t = wpool.tile([128, CPJ * BC], BF, tag=f"ab{c}", name=f"ab{c}")
                nc.sync.dma_start(
                    t[:], ab.ap()[:, c * CPJ * BC : (c + 1) * CPJ * BC]
                )
                ab_sb.append(t)
            wb_sb = wg_sb[:, 0:32]
            wu_sb = wg_sb[:, 32:64]
            g0_sb = wg_sb[0:120, 64 : 64 + P]
            g1_sb = wg_sb[0:120, 64 + P : 64 + 2 * P]
            g2_sb = wg_sb[0:72, 64 + 2 * P : 64 + 3 * P]

            qt0 = wpool.tile([120, I * BC], BF, tag="qt0")
            qt1 = wpool.tile([120, I * BC], BF, tag="qt1")
            qtu = wpool.tile([72, I * BC], BF, tag="qtu")
            sgu = wpool.tile([96, 3 * BC], BF, tag="sgu")
            sgA = wpool.tile([96, 5 * BC], BF, tag="sgA")
            sgB = wpool.tile([96, 5 * BC], BF, tag="sgB")

            nev = [0]

            def evac(dst, src):
                if nev[0] % 2 == 0:
                    nc.scalar.copy(dst, src)
                else:
                    nc.vector.tensor_copy(dst, src)
                nev[0] += 1

            nasm = [0]

            def assemble(qtc, sgt, scrt, ng):
                """sg -> DRAM scratch (scatter) -> qt (flat reload)."""
                h1src = sgt[:].rearrange("p (g b) -> p g b", g=ng)
                h1dst = scrt.ap().rearrange("g (p b) -> p g b", p=96)
                eng = nc.sync if nasm[0] % 2 == 0 else nc.scalar
                h1 = eng.dma_start(h1dst, h1src)
                tc.dep_state.set_after_insts(qtc.tensor.name, h1.ins)
                h2src = scrt.ap().rearrange(
                    "g (lud ib) -> (g lud) ib", lud=24
                )
                eng.dma_start(qtc[:], h2src)
                nasm[0] += 1

            # ---- stage-1 unary' (runs first: warms the PE) ----
            for h in range(3):
                pu = psu.tile([96, BC], FP, tag="pu")
                for ms in range(3):
                    l = 3 * h + ms
                    rl = min(l, O - 1)   # h2 ms2 = dummy slot init (reuse l=7)
                    nc.tensor.matmul(
                        pu[32 * ms : 32 * ms + 32, :],
                        wu_sb,
                        au_sb[:, rl * BC : (rl + 1) * BC],
                        start=True,
                        stop=True,
                    )
                evac(sgu[:, h * BC : (h + 1) * BC], pu[:])
            assemble(qtu, sgu, scrU, 3)

            # ---- stage-1 offdiag: groups of 3 jp per psum bank ----
            for g in range(10):
                pb = psb.tile([96, BC], FP, tag="pb")
                for l in range(3):
                    jp = min(3 * g + l, NP - 1)   # g9 l1/l2 = dummy (jp27)
                    c = jp // CPJ
                    off = (jp - c * CPJ) * BC
                    nc.tensor.matmul(
                        pb[32 * l : 32 * l + 32, :],
                        wb_sb,
                        ab_sb[c][:, off : off + BC],
                        start=True,
                        stop=True,
                    )
                sgt = sgA if g < 5 else sgB
                cg = g if g < 5 else g - 5
                evac(sgt[:, cg * BC : (cg + 1) * BC], pb[:])
                if g == 4:
                    assemble(qt0, sgA, scrA, 5)
                if g == 9:
                    assemble(qt1, sgB, scrB, 5)

            # ---- stage-2: scores, min, softmax ----
            fin = mpool.tile([128, 4 * NBT], FP, tag="fin", bufs=1)
            for bt in range(NBT):
                merged = mpool.tile([128, 4], FP, tag="m")
                for i in range(I):
                    sc = pss.tile([128, P], FP, tag="sc")
                    col = i * BC + bt * 128
                    nc.tensor.matmul(
                        sc[:], qt0[:, col : col + 128], g0_sb,
                        start=True, stop=False,
                    )
                    nc.tensor.matmul(
                        sc[:], qt1[:, col : col + 128], g1_sb,
                        start=False, stop=False,
                    )
                    nc.tensor.matmul(
                        sc[:], qtu[:, col : col + 128], g2_sb,
                        start=False, stop=True,
                    )
                    nc.vector.tensor_reduce(
                        merged[:, i : i + 1], sc[:],
                        axis=mybir.AxisListType.X, op=MIN,
                    )
                mx = mpool.tile([128, 1], FP, tag="mx")
                nc.vector.tensor_reduce(
                    mx[:], merged[:], axis=mybir.AxisListType.X,
                    op=mybir.AluOpType.max,
                )
                sh = mpool.tile([128, 4], FP, tag="sh")
                nc.vector.tensor_scalar_sub(sh[:], merged[:], mx[:])
                ex = mpool.tile([128, 4], FP, tag="ex")
                sm = mpool.tile([128, 1], FP, tag="sm")
                nc.scalar.activation(
                    ex[:], sh[:], mybir.ActivationFunctionType.Exp, accum_out=sm[:]
                )
                rc = mpool.tile([128, 1], FP, tag="rc")
                nc.vector.reciprocal(rc[:], sm[:])
                pr = mpool.tile([128, 4], FP, tag="pr")
                nc.vector.tensor_scalar_mul(pr[:], ex[:], rc[:])
                pr3 = pr[:].rearrange("p (a b) -> p a b", b=2)
                nc.vector.tensor_add(
                    fin[:, bt * 4 : bt * 4 + 2], pr3[:, :, 0], pr3[:, :, 1]
                )
                nc.vector.memset(fin[:, bt * 4 + 2 : bt * 4 + 4], 0.0)
            outv = out.ap().rearrange("(a p) m -> p a m", p=128)
            nc.sync.dma_start(outv, fin[:].rearrange("p (a m) -> p a m", a=NBT))

    nc.compile()
    return nc


def _get_module():
    if "nc" not in _CACHED:
        _CACHED["nc"] = _build_module()
    return _CACHED["nc"]


def _host_inputs(unary_feats, binary_feats, rule_unary, rule_binary):
    import ml_dtypes

    bf16 = ml_dtypes.bfloat16
    uf = np.asarray(unary_feats, dtype=np.float32).astype(bf16)
    bf = np.asarray(binary_feats, dtype=np.float32).astype(bf16)
    ru = np.asarray(rule_unary, dtype=np.float32)
    rb = np.asarray(rule_binary, dtype=np.float32)

    wb = np.zeros((128, 32), np.float32)
    for u, (n, m) in enumerate(_PAIRS3):
        for d in range(2):
            for i in range(I):
                col = (u * 2 + d) * 4 + i
                fst, snd = ((n, m), (m, n)) if d == 0 else ((m, n), (n, m))
                wb[0:64, col] = rb[i, fst[0], fst[1], :]
                wb[64:128, col] = rb[i, snd[0], snd[1], :]
    wu2 = np.zeros((128, 32), np.float32)
    for v in range(V):
        for i in range(I):
            wu2[0:64, v * 4 + i] = ru[i, v, :]
            wu2[64:128, v * 4 + i] = rb[i, v, v, :]
    g0, g1, g2 = _build_g()
    wgm = np.zeros((128, 64 + 3 * P), np.float32)
    wgm[:, 0:32] = wb
    wgm[:, 32:64] = wu2
    wgm[0:120, 64 : 64 + P] = g0
    wgm[0:120, 64 + P : 64 + 2 * P] = g1
    wgm[0:72, 64 + 2 * P : 64 + 3 * P] = g2
    wgm = wgm.astype(bf16)

    J = np.array([p[0] for p in _PAIRS])
    K = np.array([p[1] for p in _PAIRS])
    dia = np.arange(O)
    in_maps = []
    for c in range(N_CORES):
        bfc = bf[c * BC : (c + 1) * BC]                    # [BC, O, O, E]
        x0 = bfc.transpose(1, 2, 3, 0)                     # [j, k, e, b]
        up = x0[J, K]                                      # [28, E, BC]
        dn = x0[K, J]
        abm = np.ascontiguousarray(
            np.concatenate([up, dn], axis=1).transpose(1, 0, 2)
        ).reshape(128, NP * BC)
        ufc = uf[c * BC : (c + 1) * BC]                    # [BC, O, E]
        ut = ufc.transpose(1, 2, 0)                        # [l, e, b]
        dg = bfc[:, dia, dia, :].transpose(1, 2, 0)        # [l, e, b]
        aum = np.ascontiguousarray(
            np.concatenate([ut, dg], axis=1).transpose(1, 0, 2)
        ).reshape(128, O * BC)
        in_maps.append({"ab": abm, "au": aum, "wg": wgm})
    return in_maps


TRACE = False  # set True (e.g. from test.py) to capture an NTFF profile


def kernel(unary_feats, binary_feats, rule_unary, rule_binary):
    from concourse.bass_utils import run_bass_kernel_spmd

    nc = _get_module()
    in_maps = _host_inputs(unary_feats, binary_feats, rule_unary, rule_binary)
    res = run_bass_kernel_spmd(
        nc, in_maps, core_ids=list(range(N_CORES)), trace=TRACE
    )
    _CACHED["last_results"] = res
    return np.concatenate(
        [res.results[c]["out"] for c in range(N_CORES)], axis=0
    )
